# revision 24
# baseline (speedup 1.0000x reference)
"""Trainium2 Bass kernel for nn_CustomNLLLoss (binary-class NLL with per-class means).

Math: for C=2, log_softmax picked value obeys
    -picked_i = softplus(x1-x0) if t=0 else softplus(x0-x1)
With d = x1 - x0, g = softplus(d) and softplus(-d) = g - d:
    sum0 = sum_{t=0} g        = S_g - S_tg
    sum1 = sum_{t=1} (g - d)  = S_tg - S_td
    loss = sum0/n0 + sum1/n1
So each core only needs S_g, S_tg, S_td, n1 over its shard — 4 scalars,
combined on the host.

Per-core device work (M = 1M samples as [128 partitions x 8192]):
    DVE : d = x1 - x0 (strided sub), 2 fused tensor_tensor_reduce (t*g, t*d)
    ACT : g = softplus(d) with accum_out => S_g ; copy(t) with accum_out => n1
    PE  : one tiny ones-matmul to fold [128,4] partials across partitions
"""

import sys

for _p in ("/opt/trn_rl_repo", "/root/.axon_site/_ro/trn_rl_repo"):
    if _p not in sys.path:
        sys.path.append(_p)

import ml_dtypes
import numpy as np

import concourse.bass as bass
import concourse.tile as tile
from concourse import mybir
import concourse.bass_isa as bass_isa
from concourse.bass_utils import run_bass_kernel_spmd

N_CORES = 8
N = 8388608
M = N // N_CORES      # samples per core
P = 128               # SBUF partitions
Q = M // P            # per-partition samples per core (8192)
F = 1024              # per-partition samples per tile
T = Q // F            # tiles per core

f32 = mybir.dt.float32
bf16 = mybir.dt.bfloat16

# per-partition (offset, size) chunks: last full tile split in two to halve
# the serial sub->exp->ln->stt tail after the final DMA lands
CHUNKS = [(i * F, F) for i in range(T - 1)] + [
    ((T - 1) * F, F // 2),
    ((T - 1) * F + F // 2, F // 2),
]


def _legalize_waits(nc, max_waits=1):
    """This walrus build rejects instructions carrying more than ~1 sync
    wait ("Too many sync wait commands"), but Tile's Rust wait-assigner
    happily attaches several. Hoist excess waits onto same-engine NOPs
    inserted immediately before the instruction — sequencers execute waits
    in program order, so semantics are unchanged."""
    n = 0
    for f in nc.m.functions:
        for blk in f.blocks:
            il = blk.instructions
            i = 0
            while i < len(il):
                inst = il[i]
                si = getattr(inst, "sync_info", None)
                if si is not None and len(si.on_wait) > max_waits:
                    waits = list(si.on_wait)
                    extra, keep = waits[:-max_waits], waits[-max_waits:]
                    nops = []
                    for w in extra:
                        n += 1
                        nops.append(mybir.InstNoOp(
                            name=f"I-waitfix-{n}",
                            sync_info=mybir.SyncInfo(on_wait=[w], on_update=[]),
                            bass_nofuse=True,
                            engine=inst.engine,
                        ))
                    inst.sync_info = mybir.SyncInfo(
                        on_wait=keep, on_update=list(si.on_update)
                    )
                    il[i:i] = nops
                    i += len(nops)
                i += 1
    return nc


def build_nc():
    nc = bass.Bass("TRN2")
    xs = nc.declare_dram_parameter("xs", [P, Q, 2], f32, isOutput=False)
    ts = nc.declare_dram_parameter("ts", [P, Q], bf16, isOutput=False)
    out = nc.declare_dram_parameter("out", [P, 4], f32, isOutput=True)
    NT = len(CHUNKS)

    with tile.TileContext(nc) as tc:
        with (
            tc.tile_pool(name="io", bufs=NT) as iop,
            tc.tile_pool(name="wk", bufs=2) as wp,
            tc.tile_pool(name="st", bufs=1) as sp,
        ):
            stats_g = sp.tile([P, NT], f32)
            stats_tg = sp.tile([P, NT], f32)
            stats_td = sp.tile([P, NT], f32)
            stats_t = sp.tile([P, NT], f32)

            for i, (o0, sz) in enumerate(CHUNKS):
                xt = iop.tile([P, sz, 2], f32, tag="x")
                tt = iop.tile([P, sz], bf16, tag="t")
                nc.sync.dma_start(out=xt, in_=xs[:, o0 : o0 + sz, :])
                nc.sync.dma_start(out=tt, in_=ts[:, o0 : o0 + sz])

                # Order keeps cross-engine waits at <=1 per instruction
                # (ISA wait-slot limit): td-STT syncs DVE on the t DMA before
                # tg-STT, whose only new dep is then the ACT-produced g.
                # 1-element DVE read of tt: syncs the DVE clock on the t DMA
                # here (TensorCopy has spare wait slots) so the STT below —
                # whose ISA struct encodes only ONE sync wait — doesn't need
                # both its own-engine pipeline wait and a DMA wait.
                touch = wp.tile([1, 1], f32, tag="touch")
                nc.vector.tensor_copy(out=touch, in_=tt[0:1, 0:1])
                d = wp.tile([P, sz], f32, tag="d")
                nc.vector.tensor_tensor(
                    out=d, in0=xt[:, :, 1], in1=xt[:, :, 0],
                    op=mybir.AluOpType.subtract,
                )
                s2 = wp.tile([P, sz], f32, tag="s2")
                nc.vector.scalar_tensor_tensor(
                    out=s2, in0=tt, scalar=1.0, in1=d,
                    op0=mybir.AluOpType.mult, op1=mybir.AluOpType.mult,
                    accum_out=stats_td[:, i : i + 1],
                )
                e = wp.tile([P, sz], f32, tag="e")
                nc.scalar.activation(
                    out=e, in_=d, func=mybir.ActivationFunctionType.Exp,
                )
                g = wp.tile([P, sz], f32, tag="g")
                nc.scalar.activation(
                    out=g, in_=e,
                    func=mybir.ActivationFunctionType.Ln,
                    bias=1.0, scale=1.0,
                    accum_out=stats_g[:, i : i + 1],
                )
                s1 = wp.tile([P, sz], f32, tag="s1")
                nc.vector.scalar_tensor_tensor(
                    out=s1, in0=tt, scalar=1.0, in1=g,
                    op0=mybir.AluOpType.mult, op1=mybir.AluOpType.mult,
                    accum_out=stats_tg[:, i : i + 1],
                )
                tc_scr = wp.tile([P, sz], f32, tag="tc")
                nc.scalar.activation(
                    out=tc_scr, in_=tt,
                    func=mybir.ActivationFunctionType.Copy,
                    accum_out=stats_t[:, i : i + 1],
                )

            red = sp.tile([P, 4], f32)
            for j, st in enumerate((stats_g, stats_tg, stats_td, stats_t)):
                nc.vector.tensor_reduce(
                    out=red[:, j : j + 1], in_=st,
                    axis=mybir.AxisListType.X, op=mybir.AluOpType.add,
                )
            nc.sync.dma_start(out=out[:, :], in_=red)
    return _legalize_waits(nc)


_NC = None


def get_nc():
    global _NC
    if _NC is None:
        _NC = build_nc()
    return _NC


def run_device(x, tb, **spmd_kwargs):
    """x: [N,2] f32 contiguous, tb: [N] bfloat16. Returns (sums[4] float64, raw results)."""
    in_maps = []
    for c in range(N_CORES):
        in_maps.append({
            "xs": x[c * M : (c + 1) * M].reshape(P, Q, 2),
            "ts": tb[c * M : (c + 1) * M].reshape(P, Q),
        })
    res = run_bass_kernel_spmd(get_nc(), in_maps, list(range(N_CORES)), **spmd_kwargs)
    per_core = np.stack([r["out"] for r in res.results]).astype(np.float64)
    return per_core.sum(axis=(0, 1)), res


def kernel(x, targets):
    x = np.ascontiguousarray(np.asarray(x), dtype=np.float32)
    tb = np.asarray(targets).astype(ml_dtypes.bfloat16)  # 0/1 exact in bf16
    (s_g, s_tg, s_td, n1), _ = run_device(x, tb)
    sum0 = s_g - s_tg
    sum1 = s_tg - s_td
    n0 = float(N) - n1
    p = sum0 / n0 if n0 > 0 else 0.0
    r = sum1 / n1 if n1 > 0 else 0.0
    return np.array(p + r, dtype=np.float32)
